# revision 1
# baseline (speedup 1.0000x reference)
"""Trainium2 Bass kernel for nn_Decoder_10230612099842.

2-layer decoder (rmsnorm / qkv+RoPE / causal attention / o-proj / rmsnorm /
silu-gated MLP / final rmsnorm) over a ragged-chunk-expanded input.

Strategy (8 NeuronCores = one TRN2 chip):
  - Host: ragged chunk expansion (searchsorted gather) + residual add, weight
    norm-folding, bf16 cast, head-dim pair-permutation for RoPE, per-core
    slicing.
  - Device: data-parallel over batch (2 groups of 4 cores), sequence-parallel
    over tokens within a group (512 tokens/core). Activations live
    feature-major [D, S].  Per layer, one bf16 AllGather of k and one of v
    inside each 4-core group; attention runs a uniform slot loop per head
    with per-core causal mask inputs (mask applied multiplicatively to
    exp-scores; a ones-column appended to v makes masked slots drop out of
    both the softmax numerator and denominator).  Scores for two k-slots
    share one 2-bank PSUM tile so exp/mask run as single wide ops, and the
    score matmuls are emitted one slot-pair ahead of the AV accumulation so
    the in-order PE never waits on the exp round-trip.
  - Matmuls in bf16 with fp32 PSUM accumulation; residual stream fp32.
"""

import numpy as np
import ml_dtypes
from contextlib import ExitStack

import concourse.bass as bass
import concourse.tile as tile
from concourse import bacc, mybir
from concourse.bass_utils import run_bass_kernel_spmd

F32 = mybir.dt.float32
BF16 = mybir.dt.bfloat16
AF = mybir.ActivationFunctionType

# model constants (full problem)
B, K, T, D, NH, HD, DFF, L = 2, 512, 2048, 1024, 16, 64, 4096, 2
EPS = 1e-5
G = 4  # cores per batch group

# stream_shuffle mask: swap adjacent partition pairs within each 32-quadrant
PAIR_MASK = [i ^ 1 for i in range(32)]


def build_decoder(T_, S_, D_, NH_, HD_, DFF_, L_, n_cores=8, sim_safe=False):
    """Build the SPMD decoder graph. S_ = tokens per core, T_ = total tokens
    per batch. Group size G divides cores into batch groups."""
    Dt = D_ // 128          # D partition-tiles
    QT = (NH_ * HD_) // 128  # head-pair tiles (2 heads per tile)
    KT = T_ // 128          # k-slots per head
    KP = KT // 2            # k-slot pairs
    TT = S_ // 128          # local token tiles
    CH = max(1, DFF_ // 1024)   # mlp chunks
    DFT = (DFF_ // CH) // 128   # dff tiles per chunk (8)
    WV = min(512, NH_ * HD_)    # v-proj psum width
    VH = (NH_ * HD_) // WV      # v-proj col halves
    HPV = WV // HD_             # heads per v-psum
    scale = 1.0 / float(np.sqrt(HD_))
    n_groups = n_cores // G
    rg = [list(range(g * G, (g + 1) * G)) for g in range(n_groups)]

    nc = bacc.Bacc("TRN2", target_bir_lowering=False, debug=False,
                   num_devices=n_cores)

    # ---- I/O ----
    x0T = nc.dram_tensor("x0T", [D_, S_], F32, kind="ExternalInput")
    wqk = nc.dram_tensor("wqk", [L_, 2 * QT, Dt, 128, 128], BF16,
                         kind="ExternalInput")  # packed q|k lhsT tiles
    wv = nc.dram_tensor("wv", [L_, D_, D_], BF16, kind="ExternalInput")
    wo = nc.dram_tensor("wo", [L_, Dt, QT, 128, 128], BF16,
                        kind="ExternalInput")  # packed per-dt slabs
    w13 = nc.dram_tensor("w13", [L_, CH, DFT, 2, Dt, 128, 128], BF16,
                         kind="ExternalInput")
    w2 = nc.dram_tensor("w2", [L_, CH, Dt, DFT, 128, 128], BF16,
                        kind="ExternalInput")
    cosP = nc.dram_tensor("cosP", [128, S_], BF16, kind="ExternalInput")
    sinP = nc.dram_tensor("sinP", [128, S_], BF16, kind="ExternalInput")
    masks = nc.dram_tensor("masks", [KP, 128, 2 * S_], BF16,
                           kind="ExternalInput")  # slot-pair packed
    fw = nc.dram_tensor("fw", [128, Dt], F32, kind="ExternalInput")
    out = nc.dram_tensor("out", [D_, S_], F32, kind="ExternalOutput")

    with tile.TileContext(nc) as tc, ExitStack() as ctx:
        # ---- pools ----
        singles = ctx.enter_context(tc.tile_pool(name="singles", bufs=1))
        wqk_p = ctx.enter_context(tc.tile_pool(name="wqk", bufs=3))
        wv_p = ctx.enter_context(tc.tile_pool(name="wv", bufs=Dt))
        wo_p = ctx.enter_context(tc.tile_pool(name="wo", bufs=3))
        w13_p = ctx.enter_context(tc.tile_pool(name="w13", bufs=3))
        w2_p = ctx.enter_context(tc.tile_pool(name="w2", bufs=3))
        h_p = ctx.enter_context(tc.tile_pool(name="h", bufs=Dt))
        q_p = ctx.enter_context(tc.tile_pool(name="q", bufs=QT))
        vstg_p = ctx.enter_context(tc.tile_pool(name="vstg", bufs=TT))
        kslab_p = ctx.enter_context(tc.tile_pool(name="kslab", bufs=2))
        vslab_p = ctx.enter_context(tc.tile_pool(name="vslab", bufs=3))
        e_p = ctx.enter_context(tc.tile_pool(name="e", bufs=6))
        tmp_p = ctx.enter_context(tc.tile_pool(name="tmp", bufs=2))
        oh_p = ctx.enter_context(tc.tile_pool(name="oh", bufs=QT))
        avs_p = ctx.enter_context(tc.tile_pool(name="avs", bufs=4))
        prod_p = ctx.enter_context(tc.tile_pool(name="prod", bufs=DFT + 1))
        oacc_p = ctx.enter_context(tc.tile_pool(name="oacc", bufs=Dt))
        sq_p = ctx.enter_context(tc.tile_pool(name="sq", bufs=3))
        small_p = ctx.enter_context(tc.tile_pool(name="small", bufs=2))
        ps_big = ctx.enter_context(tc.tile_pool(name="ps_big", bufs=3,
                                                space="PSUM"))
        ps_av = ctx.enter_context(tc.tile_pool(name="ps_av", bufs=2,
                                               space="PSUM"))
        dram = ctx.enter_context(tc.tile_pool(name="dram", bufs=2,
                                              space="DRAM"))

        # ---- persistent constants ----
        ones_col = singles.tile([128, 1], BF16, tag="ones_col")
        nc.vector.memset(ones_col[:], 1.0)
        eps_sb = singles.tile([1, 1], F32, tag="eps")
        nc.vector.memset(eps_sb[:], EPS)
        ones_row = singles.tile([1, 128], BF16, tag="ones_row")
        nc.vector.memset(ones_row[:], 1.0)
        cos_sb = singles.tile([128, S_], BF16, tag="cos")
        nc.sync.dma_start(cos_sb[:], cosP[:])
        sin_sb = singles.tile([128, S_], BF16, tag="sin")
        nc.sync.dma_start(sin_sb[:], sinP[:])
        fw_sb = singles.tile([128, Dt], F32, tag="fw")
        nc.sync.dma_start(fw_sb[:], fw[:])
        mask_sb = []
        for p in range(KP):
            m = singles.tile([128, 2 * S_], BF16, tag=f"mask{p}")
            nc.sync.dma_start(m[:], masks[p])
            mask_sb.append(m)
        x_sb = []
        for t in range(Dt):
            xt = singles.tile([128, S_], F32, tag=f"x{t}")
            nc.sync.dma_start(xt[:], x0T[t * 128:(t + 1) * 128, :])
            x_sb.append(xt)

        def rmsnorm_bcast(x_tiles):
            """Return a PSUM tile [128, S] holding rstd broadcast to all
            partitions (1/sqrt(mean(x^2)+eps) per token column)."""
            ssum = ps_av.tile([128, S_], F32, tag="av")
            for t in range(Dt):
                sq = sq_p.tile([128, S_], BF16, tag="sq")
                nc.vector.tensor_mul(sq[:], x_tiles[t][:], x_tiles[t][:])
                nc.tensor.matmul(ssum[0:1, :], ones_col[:], sq[:],
                                 start=(t == 0), stop=(t == Dt - 1))
            rstd = small_p.tile([1, S_], F32, tag="rstd")
            nc.scalar.activation(rstd[:], ssum[0:1, :], AF.Sqrt,
                                 bias=eps_sb[:], scale=1.0 / D_)
            nc.vector.reciprocal(rstd[:], rstd[:])
            rstd_bf = small_p.tile([1, S_], BF16, tag="rstd_bf")
            nc.vector.tensor_copy(rstd_bf[:], rstd[:])
            rn = ps_av.tile([128, S_], F32, tag="av")
            nc.tensor.matmul(rn[:], ones_row[:], rstd_bf[:],
                             start=True, stop=True)
            return rn

        def rmsnorm_to_h(x_tiles):
            rn = rmsnorm_bcast(x_tiles)
            hs = []
            for t in range(Dt):
                ht = h_p.tile([128, S_], BF16, tag="h")
                nc.vector.tensor_mul(ht[:], x_tiles[t][:], rn[:])
                hs.append(ht)
            return hs

        def rope_from_psum(ps, dst):
            """dst(bf16) = ps*cos + shuffle(ps)*sin  (pair-permuted RoPE)."""
            qb = tmp_p.tile([128, S_], BF16, tag="qb")
            nc.vector.tensor_copy(qb[:], ps[:])
            shuf = tmp_p.tile([128, S_], BF16, tag="shuf")
            nc.vector.stream_shuffle(shuf[:], qb[:], PAIR_MASK)
            qc = tmp_p.tile([128, S_], BF16, tag="qc")
            nc.vector.tensor_mul(qc[:], qb[:], cos_sb[:])
            nc.vector.tensor_mul(shuf[:], shuf[:], sin_sb[:])
            nc.vector.tensor_add(dst[:], qc[:], shuf[:])

        for l in range(L_):
            # ---------------- attention sublayer ----------------
            h = rmsnorm_to_h(x_sb)

            # k tiles first -> rope -> stage -> AllGather per head-half so
            # attention on the first heads can start while the rest is in
            # flight
            NHH = NH_ // 2
            QH = QT // 2
            kin_k = dram.tile([NH_, HD_, S_], BF16, tag="kin_k")
            kout_ks = []
            for ck in range(2):
                for j in range(ck * QH, (ck + 1) * QH):
                    wj = wqk_p.tile([128, Dt * 128], BF16, tag="wqk")
                    nc.gpsimd.dma_start(
                        wj.rearrange("p (k c) -> p k c", c=128),
                        wqk[l, QT + j].rearrange("k p c -> p k c"))
                    ps = ps_big.tile([128, 2 * S_], F32, tag="big")
                    for kt in range(Dt):
                        nc.tensor.matmul(ps[:, 0:S_],
                                         wj[:, kt * 128:(kt + 1) * 128],
                                         h[kt][:], start=(kt == 0),
                                         stop=(kt == Dt - 1))
                    kj = tmp_p.tile([128, S_], BF16, tag="kj")
                    rope_from_psum(ps[:, 0:S_], kj)
                    nc.sync.dma_start(
                        kin_k[2 * j:2 * j + 2].rearrange("h d s -> (h d) s"),
                        kj[:])
                ko = dram.tile([G, NHH, HD_, S_], BF16, tag=f"kout_k{ck}")
                nc.gpsimd.collective_compute(
                    "AllGather", mybir.AluOpType.bypass, replica_groups=rg,
                    ins=[kin_k[ck * NHH:(ck + 1) * NHH].opt()],
                    outs=[ko.opt()])
                kout_ks.append(ko)

            # v tiles (token-major, no ones column: the receiver-side slab
            # carries a 64-wide ones block instead) -> AllGather per half
            wv_sb = []
            for kt in range(Dt):
                wt = wv_p.tile([128, D_], BF16, tag="wv")
                nc.gpsimd.dma_start(wt[:], wv[l, kt * 128:(kt + 1) * 128, :])
                wv_sb.append(wt)
            kin_v = dram.tile([NH_, S_, HD_], BF16, tag="kin_v")
            kout_vs = []
            for half in range(VH):
                for tt in range(TT):
                    ps = ps_big.tile([128, 2 * S_], F32, tag="big")
                    for kt in range(Dt):
                        nc.tensor.matmul(
                            ps[:, 0:WV], h[kt][:, tt * 128:(tt + 1) * 128],
                            wv_sb[kt][:, half * WV:(half + 1) * WV],
                            start=(kt == 0), stop=(kt == Dt - 1))
                    vs = vstg_p.tile([128, WV], BF16, tag="vstg")
                    nc.vector.tensor_copy(vs[:], ps[:, 0:WV])
                    nc.sync.dma_start(
                        kin_v[half * HPV:(half + 1) * HPV].rearrange(
                            "h s c -> s h c")[tt * 128:(tt + 1) * 128],
                        vs.rearrange("p (h c) -> p h c", c=HD_))
                vo = dram.tile([G, HPV, S_, HD_], BF16, tag=f"kout_v{half}")
                nc.gpsimd.collective_compute(
                    "AllGather", mybir.AluOpType.bypass, replica_groups=rg,
                    ins=[kin_v[half * HPV:(half + 1) * HPV].opt()],
                    outs=[vo.opt()])
                kout_vs.append(vo)

            # q tiles (feature-major, rope'd) — overlaps the AllGathers
            q_sb = []
            for j in range(QT):
                wj = wqk_p.tile([128, Dt * 128], BF16, tag="wqk")
                nc.gpsimd.dma_start(wj.rearrange("p (k c) -> p k c", c=128),
                                    wqk[l, j].rearrange("k p c -> p k c"))
                ps = ps_big.tile([128, 2 * S_], F32, tag="big")
                for kt in range(Dt):
                    nc.tensor.matmul(ps[:, 0:S_],
                                     wj[:, kt * 128:(kt + 1) * 128],
                                     h[kt][:], start=(kt == 0),
                                     stop=(kt == Dt - 1))
                qj = q_p.tile([128, S_], BF16, tag="q")
                rope_from_psum(ps[:, 0:S_], qj)
                q_sb.append(qj)

            # attention, one head PAIR at a time; two k-slots share one
            # 2-bank PSUM tile so exp/mask are single wide ops; score matmuls
            # run one slot-pair ahead of the AV accumulation
            oh_sb = []
            for j in range(QT):
                kpair = kslab_p.tile([128, KT * 128], BF16, tag="kslab")
                for hh in range(2):
                    head = 2 * j + hh
                    ko = kout_ks[head // NHH]
                    nc.sync.dma_start(
                        kpair[hh * 64:(hh + 1) * 64, :].rearrange(
                            "d (g s) -> d g s", g=G),
                        ko[:, head % NHH].rearrange("g d s -> d g s"))
                vsl = []
                for hh in range(2):
                    head = 2 * j + hh
                    vo = kout_vs[head // HPV]
                    # slot layout [v(64) | ones(64)]: AV rows 64-127 become
                    # the softmax denominator replicated across partitions
                    vslab = vslab_p.tile([128, KT, 128], BF16, tag="vslab",
                                         name=f"vslab{j}_{hh}")
                    for g in range(G):
                        nc.sync.dma_start(
                            vslab[:, g * TT:(g + 1) * TT, 0:HD_],
                            vo[g, head % HPV].rearrange(
                                "(t p) c -> p t c", p=128))
                    nc.gpsimd.memset(vslab[:, :, HD_:128], 1.0)
                    vsl.append(vslab)
                av = [ps_av.tile([128, S_], F32, tag="av", name=f"av{j}_{x}")
                      for x in range(2)]
                es_q = []
                for p in range(KP):
                    es = []
                    for hh in range(2):
                        sc2 = ps_big.tile([128, 2 * S_], F32, tag="big")
                        for u in range(2):
                            s = 2 * p + u
                            nc.tensor.matmul(
                                sc2[:, u * S_:(u + 1) * S_],
                                kpair[hh * 64:hh * 64 + 64,
                                      s * 128:(s + 1) * 128],
                                q_sb[j][hh * 64:hh * 64 + 64, :],
                                start=True, stop=True)
                        e2 = e_p.tile([128, 2 * S_], BF16, tag="e")
                        nc.scalar.activation(e2[:], sc2[:], AF.Exp,
                                             scale=scale)
                        nc.vector.tensor_mul(e2[:], e2[:], mask_sb[p][:])
                        es.append(e2)
                    es_q.append(es)
                    LEAD = min(2, KP - 1)
                    if p >= LEAD:
                        p2 = p - LEAD
                        for hh in range(2):
                            for u in range(2):
                                s = 2 * p2 + u
                                nc.tensor.matmul(
                                    av[hh][:], vsl[hh][:, s, :],
                                    es_q[p2][hh][:, u * S_:(u + 1) * S_],
                                    start=(s == 0), stop=(s == KT - 1))
                for p2 in range(KP - min(2, KP - 1), KP):
                    for hh in range(2):
                        for u in range(2):
                            s = 2 * p2 + u
                            nc.tensor.matmul(
                                av[hh][:], vsl[hh][:, s, :],
                                es_q[p2][hh][:, u * S_:(u + 1) * S_],
                                start=(s == 0), stop=(s == KT - 1))
                # av rows 0-63 = numerator, rows 64-127 = denominator
                # replicated; normalize with a 64-lane reciprocal + mul
                ohp = oh_p.tile([128, S_], BF16, tag="oh")
                oh_sb.append(ohp)
                for hh in range(2):
                    rc = avs_p.tile([64, S_], F32, tag="rc",
                                    name=f"rc{j}_{hh}")
                    nc.vector.reciprocal(rc[:], av[hh][64:128, :])
                    # odd head writes the upper partition half (64-ch DVE
                    # ops may target either half)
                    nc.vector.tensor_mul(ohp[hh * 64:hh * 64 + 64, :],
                                         av[hh][0:64, :], rc[:])

            # o-projection + residual (K=128 per head pair)
            for dt in range(Dt):
                wos = wo_p.tile([128, QT * 128], BF16, tag="wo")
                nc.gpsimd.dma_start(wos.rearrange("p (j c) -> p j c", c=128),
                                    wo[l, dt].rearrange("j p c -> p j c"))
                ps = ps_big.tile([128, 2 * S_], F32, tag="big")
                for jp in range(QT):
                    nc.tensor.matmul(ps[:, 0:S_],
                                     wos[:, jp * 128:(jp + 1) * 128],
                                     oh_sb[jp][:], start=(jp == 0),
                                     stop=(jp == QT - 1))
                nc.vector.tensor_add(x_sb[dt][:], x_sb[dt][:], ps[:, 0:S_])

            # ---------------- mlp sublayer ----------------
            h2 = rmsnorm_to_h(x_sb)
            oacc = []
            for ch in range(CH):
                prods = []
                for df in range(DFT):
                    w13s = w13_p.tile([128, 2 * Dt * 128], BF16, tag="w13")
                    nc.gpsimd.dma_start(
                        w13s.rearrange("p (u k c) -> p u k c", u=2, c=128),
                        w13[l, ch, df].rearrange("u k p c -> p u k c"))
                    gu = ps_big.tile([128, 2 * S_], F32, tag="big")
                    for kt in range(Dt):
                        nc.tensor.matmul(gu[:, 0:S_],
                                         w13s[:, kt * 128:(kt + 1) * 128],
                                         h2[kt][:], start=(kt == 0),
                                         stop=(kt == Dt - 1))
                    for kt in range(Dt):
                        off = Dt * 128
                        nc.tensor.matmul(
                            gu[:, S_:2 * S_],
                            w13s[:, off + kt * 128:off + (kt + 1) * 128],
                            h2[kt][:], start=(kt == 0), stop=(kt == Dt - 1))
                    gs = e_p.tile([128, 2 * S_], BF16, tag="e")
                    pr = prod_p.tile([128, S_], BF16, tag="prod")
                    if sim_safe:
                        # CoreSim lacks Silu; sigmoid + explicit mul
                        nc.scalar.activation(gs[:, 0:S_], gu[:, 0:S_],
                                             AF.Sigmoid)
                        gg = tmp_p.tile([128, S_], BF16, tag="gg")
                        nc.vector.tensor_mul(gg[:], gs[:, 0:S_], gu[:, 0:S_])
                        nc.vector.tensor_mul(pr[:], gg[:], gu[:, S_:2 * S_])
                    else:
                        nc.scalar.activation(gs[:, 0:S_], gu[:, 0:S_],
                                             AF.Silu)
                        nc.vector.tensor_mul(pr[:], gs[:, 0:S_],
                                             gu[:, S_:2 * S_])
                    prods.append(pr)
                for dt in range(Dt):
                    w2s = w2_p.tile([128, DFT * 128], BF16, tag="w2")
                    nc.gpsimd.dma_start(
                        w2s.rearrange("p (j c) -> p j c", c=128),
                        w2[l, ch, dt].rearrange("j p c -> p j c"))
                    ps = ps_big.tile([128, 2 * S_], F32, tag="big")
                    for jj in range(DFT):
                        nc.tensor.matmul(ps[:, 0:S_],
                                         w2s[:, jj * 128:(jj + 1) * 128],
                                         prods[jj][:], start=(jj == 0),
                                         stop=(jj == DFT - 1))
                    if ch == 0:
                        oa = oacc_p.tile([128, S_], F32, tag="oacc")
                        nc.vector.tensor_copy(oa[:], ps[:, 0:S_])
                        oacc.append(oa)
                    else:
                        nc.vector.tensor_add(oacc[dt][:], oacc[dt][:],
                                             ps[:, 0:S_])
            for dt in range(Dt):
                nc.vector.tensor_add(x_sb[dt][:], x_sb[dt][:], oacc[dt][:])

        # ---------------- final rmsnorm ----------------
        rn = rmsnorm_bcast(x_sb)
        for dt in range(Dt):
            xn = tmp_p.tile([128, S_], F32, tag="xn")
            nc.vector.tensor_mul(xn[:], x_sb[dt][:], rn[:])
            nc.vector.tensor_scalar_mul(xn[:], xn[:], fw_sb[:, dt:dt + 1])
            nc.sync.dma_start(out[dt * 128:(dt + 1) * 128, :], xn[:])

    nc.compile()
    return nc


# ---------------------------------------------------------------------------
# host-side preparation
# ---------------------------------------------------------------------------

def _bf16(a):
    return np.ascontiguousarray(np.asarray(a, dtype=np.float32)).astype(
        ml_dtypes.bfloat16)


def _perm(HD_):
    """Head-dim pair permutation: slot 2i <- dim i, slot 2i+1 <- dim i+HD/2."""
    half = HD_ // 2
    p = np.empty(HD_, dtype=np.int64)
    p[0::2] = np.arange(half)
    p[1::2] = np.arange(half) + half
    return p


def prepare_in_maps(x0, cos, sin, wq, wk, wv_, wo_, anw, mnw, w1, w3, w2_,
                    fnw, T_, S_, D_, NH_, HD_, DFF_, L_, n_cores=8):
    """Build per-core input dicts. x0 is the already-expanded [B', T, D] fp32
    input (B' = n_cores // G batches)."""
    Dt = D_ // 128
    QT = (NH_ * HD_) // 128
    KT = T_ // 128
    KP = KT // 2
    CH = max(1, DFF_ // 1024)
    DFT = (DFF_ // CH) // 128
    perm = _perm(HD_)

    # fold norm weights into the consuming projections
    wq_e = anw[:, :, None] * wq      # [L, D, D]
    wk_e = anw[:, :, None] * wk
    wv_e = anw[:, :, None] * wv_
    w1_e = mnw[:, :, None] * w1      # [L, D, DFF]
    w3_e = mnw[:, :, None] * w3

    # permute q/k columns per head by `perm`
    def permute_cols(w):
        wh = w.reshape(L_, D_, NH_, HD_)
        return wh[:, :, :, perm].reshape(L_, D_, NH_ * HD_)

    wq_p = permute_cols(wq_e)
    wk_p = permute_cols(wk_e)

    # packed q|k lhsT tiles: [L, 2QT, Dt, 128, 128]
    wqk_pack = np.empty((L_, 2 * QT, Dt, 128, 128), dtype=np.float32)
    for j in range(QT):
        for kt in range(Dt):
            wqk_pack[:, j, kt] = wq_p[:, kt * 128:(kt + 1) * 128,
                                      j * 128:(j + 1) * 128]
            wqk_pack[:, QT + j, kt] = wk_p[:, kt * 128:(kt + 1) * 128,
                                           j * 128:(j + 1) * 128]
    # wo packed: [L, Dt, QT, 128, 128]; rows = o dims (head-major)
    wo_pack = np.empty((L_, Dt, QT, 128, 128), dtype=np.float32)
    for dt in range(Dt):
        for j in range(QT):
            wo_pack[:, dt, j] = wo_[:, j * 128:(j + 1) * 128,
                                    dt * 128:(dt + 1) * 128]
    # w13 packed: [L, CH, DFT, 2, Dt, 128, 128]
    csz = DFF_ // CH
    w13_pack = np.empty((L_, CH, DFT, 2, Dt, 128, 128), dtype=np.float32)
    for ch in range(CH):
        for df in range(DFT):
            c0 = ch * csz + df * 128
            for kt in range(Dt):
                w13_pack[:, ch, df, 0, kt] = w1_e[:, kt * 128:(kt + 1) * 128,
                                                  c0:c0 + 128]
                w13_pack[:, ch, df, 1, kt] = w3_e[:, kt * 128:(kt + 1) * 128,
                                                  c0:c0 + 128]
    # w2 packed: [L, CH, Dt, DFT, 128, 128]
    w2_pack = np.empty((L_, CH, Dt, DFT, 128, 128), dtype=np.float32)
    for ch in range(CH):
        for dt in range(Dt):
            for j in range(DFT):
                r0 = ch * csz + j * 128
                w2_pack[:, ch, dt, j] = w2_[:, r0:r0 + 128,
                                            dt * 128:(dt + 1) * 128]

    wqk_b = _bf16(wqk_pack)
    wv_b = _bf16(wv_e)
    wo_b = _bf16(wo_pack)
    w13_b = _bf16(w13_pack)
    w2_b = _bf16(w2_pack)
    fw_np = np.ascontiguousarray(
        np.asarray(fnw, np.float32).reshape(Dt, 128).T)  # [128, Dt]

    # rope tables, permuted + sign-baked, duplicated per head pair -> [128, T]
    cosPf = np.asarray(cos, np.float32)[:, perm].T        # [HD, T]
    sinf = np.asarray(sin, np.float32)[:, perm].T         # [HD, T]
    sign = np.where(np.arange(HD_) % 2 == 0, -1.0, 1.0)[:, None]
    sinPf = sinf * sign
    cosP2 = np.tile(cosPf, (2, 1))                        # [128, T]
    sinP2 = np.tile(sinPf, (2, 1))

    in_maps = []
    for c in range(n_cores):
        b = c // G
        r = c % G
        t0 = r * S_
        xs = np.ascontiguousarray(x0[b, t0:t0 + S_, :].T).astype(np.float32)
        mask = np.zeros((KT, 128, S_), dtype=np.float32)
        for s in range(KT):
            kg = 128 * s + np.arange(128)[:, None]
            qg = t0 + np.arange(S_)[None, :]
            mask[s] = (kg <= qg).astype(np.float32)
        mask2 = mask.reshape(KP, 2, 128, S_).transpose(0, 2, 1, 3).reshape(
            KP, 128, 2 * S_)
        in_maps.append({
            "x0T": xs,
            "wqk": wqk_b, "wv": wv_b, "wo": wo_b, "w13": w13_b, "w2": w2_b,
            "cosP": _bf16(cosP2[:, t0:t0 + S_]),
            "sinP": _bf16(sinP2[:, t0:t0 + S_]),
            "masks": mask2.astype(ml_dtypes.bfloat16),
            "fw": fw_np,
        })
    return in_maps


def expand_input(x_processed, boundaries, counts, x_residual):
    """Ragged chunk expansion: token t of batch b takes chunk
    #{boundaries[b] <= t}, plus residual."""
    xp = np.asarray(x_processed, np.float32)
    bd = np.asarray(boundaries)
    xr = np.asarray(x_residual, np.float32)
    Bn, Tn, Dn = xr.shape
    tt = np.arange(Tn)
    out = np.empty_like(xr)
    for b in range(Bn):
        idx = np.searchsorted(bd[b], tt, side="right")
        out[b] = xp[b, idx, :] + xr[b]
    return out


_NC_CACHE = {}


def _get_nc(key):
    if key not in _NC_CACHE:
        _NC_CACHE[key] = build_decoder(*key)
    return _NC_CACHE[key]


def kernel(x_processed, boundaries, counts, x_residual, cos, sin, seq_len,
           wq, wk, wv, wo, attn_norm_w, mlp_norm_w, w1, w3, w2, final_norm_w,
           _trace=False):
    S_ = T // G
    x0 = expand_input(x_processed, boundaries, counts, x_residual)
    in_maps = prepare_in_maps(
        x0, cos, sin,
        np.asarray(wq, np.float32), np.asarray(wk, np.float32),
        np.asarray(wv, np.float32), np.asarray(wo, np.float32),
        np.asarray(attn_norm_w, np.float32), np.asarray(mlp_norm_w, np.float32),
        np.asarray(w1, np.float32), np.asarray(w3, np.float32),
        np.asarray(w2, np.float32), np.asarray(final_norm_w, np.float32),
        T, S_, D, NH, HD, DFF, L, n_cores=8)
    nc = _get_nc((T, S_, D, NH, HD, DFF, L, 8))
    res = run_bass_kernel_spmd(nc, in_maps, list(range(8)), trace=_trace)
    outp = np.empty((B, T, D), dtype=np.float32)
    for c in range(8):
        b, r = c // G, c % G
        outp[b, r * S_:(r + 1) * S_, :] = res.results[c]["out"].T
    if _trace:
        kernel.last_exec_time_ns = res.exec_time_ns
        kernel.last_results = res
    return outp



# revision 17
# speedup vs baseline: 1.1900x; 1.1900x over previous
"""Trainium2 Bass kernel for nn_Decoder_10230612099842.

2-layer decoder (rmsnorm / qkv+RoPE / causal attention / o-proj / rmsnorm /
silu-gated MLP / final rmsnorm) over a ragged-chunk-expanded input.

Strategy (8 NeuronCores = one TRN2 chip):
  - Host: ragged chunk expansion (searchsorted gather) + residual add, weight
    norm-folding, bf16 cast, head-dim pair-permutation for RoPE, per-core
    slicing.
  - Device: data-parallel over batch (2 groups of 4 cores); within a group,
    sequence-parallel with a STRIDED q-tile assignment: core r owns global
    128-token q-tiles {r, r+4, r+8, r+12}.  This makes a causal-aware slot
    loop SPMD-uniform: local q-position p only attends key slots 0..4p+3
    (40 of 64 slot-tiles instead of all 64), identically shaped on every
    core; the per-core causal boundary lives in small mask INPUTS applied
    to the four "diagonal zone" slots of each position.
  - Per layer: k/v projections in channel halves with interleaved per-half
    AllGathers inside each 4-core group; gathered k/v slabs land in SBUF
    via one strided DMA per head(-pair).  Scores/exp/AV run per head with
    key-token-major psums; a [v|ones] interleaved lhsT makes AV rows 64-127
    the softmax denominator, normalized with a fast approximate reciprocal.
  - Matmuls in bf16 with fp32 PSUM accumulation; residual stream fp32.
"""

import numpy as np
import ml_dtypes
from contextlib import ExitStack

import concourse.bass as bass
import concourse.tile as tile
from concourse import bacc, mybir
from concourse.bass_utils import run_bass_kernel_spmd

F32 = mybir.dt.float32
BF16 = mybir.dt.bfloat16
AF = mybir.ActivationFunctionType

# model constants (full problem)
B, K, T, D, NH, HD, DFF, L = 2, 512, 2048, 1024, 16, 64, 4096, 2
EPS = 1e-5
G = 4  # cores per batch group

# stream_shuffle mask: swap adjacent partition pairs within each 32-quadrant
PAIR_MASK = [i ^ 1 for i in range(32)]


def build_decoder(T_, S_, D_, NH_, HD_, DFF_, L_, n_cores=8, dbg=False):
    """Build the SPMD decoder graph. S_ = tokens per core (4 strided q-tiles),
    T_ = total tokens per batch."""
    Dt = D_ // 128            # D partition-tiles (8)
    QT = (NH_ * HD_) // 128   # head-pair tiles (8)
    KT = T_ // 128            # key slots (16)
    NP = S_ // 128            # local q positions (4)
    TT = S_ // 128            # local token tiles (4)
    CH = max(1, DFF_ // 1024)     # mlp chunks (4)
    DFT = (DFF_ // CH) // 128     # dff tiles per chunk (8)
    WV = 512                  # v-proj psum width (8 heads)
    NHH = NH_ // 2            # heads per AG half (8)
    QH = QT // 2              # pairs per AG half (4)
    scale = 1.0 / float(np.sqrt(HD_))
    n_groups = n_cores // G
    rg = [list(range(g * G, (g + 1) * G)) for g in range(n_groups)]
    # causal widths per position zone: slot s in zone p=s//4 is needed by
    # positions p..3 only -> suffix cols [128*p, 512), width W[p]
    W = [S_ - 128 * p for p in range(NP)]

    nc = bacc.Bacc("TRN2", target_bir_lowering=False, debug=False,
                   num_devices=n_cores)

    # ---- I/O ----
    x0T = nc.dram_tensor("x0T", [D_, S_], F32, kind="ExternalInput")
    wqk = nc.dram_tensor("wqk", [L_, 2 * QT, Dt, 128, 128], BF16,
                         kind="ExternalInput")  # packed q|k lhsT tiles
    wv = nc.dram_tensor("wv", [L_, D_, D_], BF16, kind="ExternalInput")
    wo = nc.dram_tensor("wo", [L_, Dt, QT, 128, 128], BF16,
                        kind="ExternalInput")  # packed per-dt slabs
    w13 = nc.dram_tensor("w13", [L_, CH, DFT, 2, Dt, 128, 128], BF16,
                         kind="ExternalInput")
    w2 = nc.dram_tensor("w2", [L_, CH, Dt, DFT, 128, 128], BF16,
                        kind="ExternalInput")
    cosP = nc.dram_tensor("cosP", [128, S_], BF16, kind="ExternalInput")
    sinP = nc.dram_tensor("sinP", [128, S_], BF16, kind="ExternalInput")
    # zone masks: [part(key), p, i(tilehalf), k2, c(q)] -> slot 4p+2i+k2
    masks = nc.dram_tensor("masks", [128, NP, 2, 2, 128], BF16,
                           kind="ExternalInput")
    fw = nc.dram_tensor("fw", [128, Dt], F32, kind="ExternalInput")
    out = nc.dram_tensor("out", [D_, S_], F32, kind="ExternalOutput")
    dbg_t = {}
    if dbg:
        for nm, shp in [("dbg_h", [128, S_]), ("dbg_q", [128, S_]),
                        ("dbg_k", [128, KT * 128]),
                        ("dbg_v", [128, KT * 2 * HD_]),
                        ("dbg_e", [128, 2 * 512]), ("dbg_av", [128, S_]),
                        ("dbg_oh", [128, S_]), ("dbg_xa", [128, S_])]:
            dbg_t[nm] = nc.dram_tensor(nm, shp, F32, kind="ExternalOutput")

    with tile.TileContext(nc) as tc, ExitStack() as ctx:
        # ---- pools ----
        singles = ctx.enter_context(tc.tile_pool(name="singles", bufs=1))
        wqk_p = ctx.enter_context(tc.tile_pool(name="wqk", bufs=3))
        wv_p = ctx.enter_context(tc.tile_pool(name="wv", bufs=Dt))
        wo_p = ctx.enter_context(tc.tile_pool(name="wo", bufs=3))
        w13_p = ctx.enter_context(tc.tile_pool(name="w13", bufs=3))
        w2_p = ctx.enter_context(tc.tile_pool(name="w2", bufs=3))
        h_p = ctx.enter_context(tc.tile_pool(name="h", bufs=Dt))
        q_p = ctx.enter_context(tc.tile_pool(name="q", bufs=QT))
        vstg_p = ctx.enter_context(tc.tile_pool(name="vstg", bufs=3))
        kslab_p = ctx.enter_context(tc.tile_pool(name="kslab", bufs=3))
        vslab_p = ctx.enter_context(tc.tile_pool(name="vslab", bufs=4))
        e_p = ctx.enter_context(tc.tile_pool(name="e", bufs=6))
        tmp_p = ctx.enter_context(tc.tile_pool(name="tmp", bufs=2))
        oh_p = ctx.enter_context(tc.tile_pool(name="oh", bufs=QT))
        rc_p = ctx.enter_context(tc.tile_pool(name="rc", bufs=2))
        prod_p = ctx.enter_context(tc.tile_pool(name="prod", bufs=DFT + 1))
        sq_p = ctx.enter_context(tc.tile_pool(name="sq", bufs=3))
        small_p = ctx.enter_context(tc.tile_pool(name="small", bufs=2))
        ps_big = ctx.enter_context(tc.tile_pool(name="ps_big", bufs=3,
                                                space="PSUM"))
        ps_av = ctx.enter_context(tc.tile_pool(name="ps_av", bufs=2,
                                               space="PSUM"))
        dram = ctx.enter_context(tc.tile_pool(name="dram", bufs=2,
                                              space="DRAM"))

        # ---- persistent constants ----
        ones_col = singles.tile([128, 1], BF16, tag="ones_col")
        nc.vector.memset(ones_col[:], 1.0)
        eps_sb = singles.tile([1, 1], F32, tag="eps")
        nc.vector.memset(eps_sb[:], EPS)
        ones_row = singles.tile([1, 128], BF16, tag="ones_row")
        nc.vector.memset(ones_row[:], 1.0)
        x_sb = []
        for t in range(Dt):
            xt = singles.tile([128, S_], F32, tag=f"x{t}")
            nc.sync.dma_start(xt[:], x0T[t * 128:(t + 1) * 128, :])
            x_sb.append(xt)
        cos_sb = singles.tile([128, S_], BF16, tag="cos")
        nc.sync.dma_start(cos_sb[:], cosP[:])
        sin_sb = singles.tile([128, S_], BF16, tag="sin")
        nc.sync.dma_start(sin_sb[:], sinP[:])
        fw_sb = singles.tile([128, Dt], F32, tag="fw")
        nc.scalar.dma_start(fw_sb[:], fw[:])
        mask_sb = singles.tile([128, NP, 2, 2, 128], BF16, tag="masks")
        nc.scalar.dma_start(mask_sb[:], masks[:])

        def rmsnorm_bcast(x_tiles):
            """Return a PSUM tile [128, S] holding rstd broadcast to all
            partitions (1/sqrt(mean(x^2)+eps) per token column)."""
            ssum = ps_av.tile([128, S_], F32, tag="av")
            for t in range(Dt):
                sq = sq_p.tile([128, S_], BF16, tag="sq")
                nc.vector.tensor_mul(sq[:], x_tiles[t][:], x_tiles[t][:])
                nc.tensor.matmul(ssum[0:1, :], ones_col[:], sq[:],
                                 start=(t == 0), stop=(t == Dt - 1))
            rstd = small_p.tile([1, S_], F32, tag="rstd")
            nc.scalar.activation(rstd[:], ssum[0:1, :], AF.Sqrt,
                                 bias=eps_sb[:], scale=1.0 / D_)
            rstd2 = small_p.tile([1, S_], F32, tag="rstd2")
            nc.vector.reciprocal_approx_fast(rstd2[:], rstd[:])
            rstd_bf = small_p.tile([1, S_], BF16, tag="rstd_bf")
            nc.vector.tensor_copy(rstd_bf[:], rstd2[:])
            rn = ps_av.tile([128, S_], F32, tag="av")
            nc.tensor.matmul(rn[:], ones_row[:], rstd_bf[:],
                             start=True, stop=True)
            return rn

        def rmsnorm_to_h(x_tiles):
            rn = rmsnorm_bcast(x_tiles)
            hs = []
            for t in range(Dt):
                ht = h_p.tile([128, S_], BF16, tag="h")
                nc.vector.tensor_mul(ht[:], x_tiles[t][:], rn[:])
                hs.append(ht)
            return hs

        dumped = set()

        def dump(nm, src_ap, n):
            if not dbg or nm in dumped:
                return
            dumped.add(nm)
            dt_ = singles.tile([128, n], F32, tag=f"t_{nm}", name=f"t_{nm}")
            nc.vector.tensor_copy(dt_[:], src_ap)
            nc.sync.dma_start(dbg_t[nm][:], dt_[:])

        def rope_from_psum(ps, dst):
            """dst(bf16) = ps*cos + shuffle(ps)*sin  (pair-permuted RoPE)."""
            qb = tmp_p.tile([128, S_], BF16, tag="qb")
            nc.vector.tensor_copy(qb[:], ps[:])
            shuf = tmp_p.tile([128, S_], BF16, tag="shuf")
            nc.vector.stream_shuffle(shuf[:], qb[:], PAIR_MASK)
            qc = tmp_p.tile([128, S_], BF16, tag="qc")
            nc.vector.tensor_mul(qc[:], qb[:], cos_sb[:])
            nc.vector.tensor_mul(shuf[:], shuf[:], sin_sb[:])
            nc.vector.tensor_add(dst[:], qc[:], shuf[:])

        for l in range(L_):
            # ---------------- attention sublayer ----------------
            h = rmsnorm_to_h(x_sb)
            if l == 0:
                dump("dbg_h", h[0][:], S_)

            # wv loads early (gpsimd queue)
            wv_sb = []
            for kt in range(Dt):
                wt = wv_p.tile([128, D_], BF16, tag="wv")
                nc.gpsimd.dma_start(wt[:], wv[l, kt * 128:(kt + 1) * 128, :])
                wv_sb.append(wt)

            kin_k = dram.tile([NH_, HD_, S_], BF16, tag="kin_k")
            kin_v = dram.tile([2, S_, WV], BF16, tag="kin_v")
            kout_ks, kout_vs = [], []
            for ck in range(2):
                # k pairs of this half -> rope -> stage -> AllGather
                for j in range(ck * QH, (ck + 1) * QH):
                    wj = wqk_p.tile([128, Dt * 128], BF16, tag="wqk")
                    nc.gpsimd.dma_start(
                        wj.rearrange("p (k c) -> p k c", c=128),
                        wqk[l, QT + j].rearrange("k p c -> p k c"))
                    ps = ps_big.tile([128, 2, 512], F32, tag="big")
                    for kt in range(Dt):
                        nc.tensor.matmul(ps[:, 0, :],
                                         wj[:, kt * 128:(kt + 1) * 128],
                                         h[kt][:], start=(kt == 0),
                                         stop=(kt == Dt - 1))
                    kj = tmp_p.tile([128, S_], BF16, tag="kj")
                    rope_from_psum(ps[:, 0, :], kj)
                    nc.sync.dma_start(
                        kin_k[2 * j:2 * j + 2].rearrange("h d s -> (h d) s"),
                        kj[:])
                ko = dram.tile([G, NHH, HD_, S_], BF16, tag=f"kout_k{ck}")
                nc.gpsimd.collective_compute(
                    "AllGather", mybir.AluOpType.bypass, replica_groups=rg,
                    ins=[kin_k[ck * NHH:(ck + 1) * NHH].opt()],
                    outs=[ko.opt()])
                kout_ks.append(ko)

                # v half (channels ck*512..): token-major -> AllGather
                for tt in range(TT):
                    ps = ps_big.tile([128, 2, 512], F32, tag="big")
                    for kt in range(Dt):
                        nc.tensor.matmul(
                            ps[:, 0, :], h[kt][:, tt * 128:(tt + 1) * 128],
                            wv_sb[kt][:, ck * WV:(ck + 1) * WV],
                            start=(kt == 0), stop=(kt == Dt - 1))
                    vs = vstg_p.tile([128, WV], BF16, tag="vstg")
                    nc.vector.tensor_copy(vs[:], ps[:, 0, :])
                    nc.sync.dma_start(
                        kin_v[ck, tt * 128:(tt + 1) * 128, :], vs[:])
                vo = dram.tile([G, S_, WV], BF16, tag=f"kout_v{ck}")
                nc.gpsimd.collective_compute(
                    "AllGather", mybir.AluOpType.bypass, replica_groups=rg,
                    ins=[kin_v[ck].opt()], outs=[vo.opt()])
                kout_vs.append(vo)

            # q tiles (feature-major, rope'd) — overlaps the AllGathers
            q_sb = []
            for j in range(QT):
                wj = wqk_p.tile([128, Dt * 128], BF16, tag="wqk")
                nc.gpsimd.dma_start(wj.rearrange("p (k c) -> p k c", c=128),
                                    wqk[l, j].rearrange("k p c -> p k c"))
                ps = ps_big.tile([128, 2, 512], F32, tag="big")
                for kt in range(Dt):
                    nc.tensor.matmul(ps[:, 0, :],
                                     wj[:, kt * 128:(kt + 1) * 128],
                                     h[kt][:], start=(kt == 0),
                                     stop=(kt == Dt - 1))
                qj = q_p.tile([128, S_], BF16, tag="q")
                rope_from_psum(ps[:, 0, :], qj)
                q_sb.append(qj)
            if l == 0:
                dump("dbg_q", q_sb[0][:], S_)

            # attention, one head PAIR at a time.  Slot s = 4p+2i+k2 of the
            # gathered keys is processed against the causal column suffix
            # [128p, 512); zone masks (per-core data) cut the diagonal.
            oh_sb = []
            for j in range(QT):
                half = j // QH
                ko = kout_ks[half]
                vo = kout_vs[half]
                hp = 2 * (j % QH)
                # k slab [128(2h x 64d), slot(=4t+r), c], one dma per src core
                kslab = kslab_p.tile([128, KT, 128], BF16, tag="kslab")
                kre = kslab.rearrange("d (t r) c -> d t r c", r=G)
                for r in range(G):
                    nc.gpsimd.dma_start(
                        kre[:, :, r, :],
                        ko[r, hp:hp + 2].rearrange(
                            "h d (t c) -> (h d) t c", c=128))
                # v slabs per head: [128(tok), slot, {v,ones}, 64]
                vsl = []
                for hh in range(2):
                    head = 2 * j + hh
                    c0 = (head % NHH) * HD_
                    vt = vslab_p.tile([128, KT, 2, HD_], BF16, tag="vslab",
                                      name=f"vsl{j}_{hh}")
                    vre = vt.rearrange("p (t r) u c -> p t r u c", r=G)
                    for r in range(G):
                        nc.sync.dma_start(
                            vre[:, :, r, 0, :],
                            vo[r, :, c0:c0 + HD_].rearrange(
                                "(t p) c -> p t c", p=128))
                    nc.gpsimd.memset(vt[:, :, 1, :], 1.0)
                    vsl.append(vt)
                if l == 0 and j == 0:
                    dump("dbg_k", kslab.rearrange("d t c -> d (t c)"),
                         KT * 128)
                    dump("dbg_v",
                         vsl[0].rearrange("p t u c -> p (t u c)"),
                         KT * 2 * HD_)
                av = [ps_av.tile([128, S_], F32, tag="av", name=f"av{j}_{x}")
                      for x in range(2)]
                # zone tiles: (p, i) x hh, each [128, 2 slots, 512]
                tiles = [(p, i) for p in range(NP) for i in range(2)]
                es = {}
                LEAD = 2

                def emit_av(idx):
                    p, i = tiles[idx]
                    for hh in range(2):
                        for k2 in range(2):
                            s = 4 * p + 2 * i + k2
                            nc.tensor.matmul(
                                av[hh][:, 128 * p:S_],
                                vsl[hh][:, s, :, :],
                                es[(p, i, hh)][:, k2, 0:W[p]],
                                start=(s == 0), stop=(s == KT - 1))

                for idx, (p, i) in enumerate(tiles):
                    for hh in range(2):
                        sc = ps_big.tile([128, 2, 512], F32, tag="big")
                        for k2 in range(2):
                            s = 4 * p + 2 * i + k2
                            nc.tensor.matmul(
                                sc[:, k2, 0:W[p]],
                                kslab[hh * 64:hh * 64 + 64,
                                      s, :],
                                q_sb[j][hh * 64:hh * 64 + 64, 128 * p:S_],
                                start=True, stop=True)
                        e2 = e_p.tile([128, 2, 512], BF16, tag="e")
                        nc.scalar.activation(e2[:, :, 0:W[p]],
                                             sc[:, :, 0:W[p]], AF.Exp,
                                             scale=scale)
                        # diagonal-zone mask (per-core data): first 128 cols
                        nc.vector.tensor_mul(e2[:, :, 0:128],
                                             e2[:, :, 0:128],
                                             mask_sb[:, p, i])
                        es[(p, i, hh)] = e2
                        if l == 0 and j == 0 and p == 0 and i == 0 \
                                and hh == 0:
                            dump("dbg_e",
                                 e2.rearrange("p a b -> p (a b)"), 1024)
                    if idx >= LEAD:
                        emit_av(idx - LEAD)
                for idx in range(len(tiles) - LEAD, len(tiles)):
                    emit_av(idx)
                # av rows 0-63 = numerator, 64-127 = denominator replicated
                if l == 0 and j == 0:
                    dump("dbg_av", av[0][:], S_)
                ohp = oh_p.tile([128, S_], BF16, tag="oh")
                oh_sb.append(ohp)
                for hh in range(2):
                    dn = rc_p.tile([64, S_], F32, tag="dn",
                                   name=f"dn{j}_{hh}")
                    nc.vector.tensor_copy(dn[:], av[hh][64:128, :])
                    rc = rc_p.tile([64, S_], F32, tag="rc",
                                   name=f"rc{j}_{hh}")
                    nc.vector.reciprocal_approx_fast(rc[:], dn[:])
                    nc.vector.tensor_mul(ohp[hh * 64:hh * 64 + 64, :],
                                         av[hh][0:64, :], rc[:])
                if l == 0 and j == 0:
                    dump("dbg_oh", ohp[:], S_)

            # o-projection + residual, two 4-pair chunks
            for oc in range(2):
                for dt in range(Dt):
                    wos = wo_p.tile([128, QH * 128], BF16, tag="wo")
                    nc.gpsimd.dma_start(
                        wos.rearrange("p (j c) -> p j c", c=128),
                        wo[l, dt, oc * QH:(oc + 1) * QH].rearrange(
                            "j p c -> p j c"))
                    ps = ps_av.tile([128, S_], F32, tag="av")
                    for jj in range(QH):
                        jp = oc * QH + jj
                        nc.tensor.matmul(ps[:],
                                         wos[:, jj * 128:(jj + 1) * 128],
                                         oh_sb[jp][:], start=(jj == 0),
                                         stop=(jj == QH - 1))
                    nc.vector.tensor_add(x_sb[dt][:], x_sb[dt][:], ps[:])
            if l == 0:
                dump("dbg_xa", x_sb[0][:], S_)

            # ---------------- mlp sublayer ----------------
            h2 = rmsnorm_to_h(x_sb)
            for ch in range(CH):
                prods = []
                for df in range(DFT):
                    w13s = w13_p.tile([128, 2 * Dt * 128], BF16, tag="w13")
                    nc.gpsimd.dma_start(
                        w13s.rearrange("p (u k c) -> p u k c", u=2, c=128),
                        w13[l, ch, df].rearrange("u k p c -> p u k c"))
                    gu = ps_big.tile([128, 2, 512], F32, tag="big")
                    for kt in range(Dt):
                        nc.tensor.matmul(gu[:, 0, :],
                                         w13s[:, kt * 128:(kt + 1) * 128],
                                         h2[kt][:], start=(kt == 0),
                                         stop=(kt == Dt - 1))
                    for kt in range(Dt):
                        off = Dt * 128
                        nc.tensor.matmul(
                            gu[:, 1, :],
                            w13s[:, off + kt * 128:off + (kt + 1) * 128],
                            h2[kt][:], start=(kt == 0), stop=(kt == Dt - 1))
                    gs = e_p.tile([128, 2, 512], BF16, tag="e")
                    pr = prod_p.tile([128, S_], BF16, tag="prod")
                    nc.scalar.activation(gs[:, 0, :], gu[:, 0, :], AF.Silu)
                    nc.vector.tensor_mul(pr[:], gs[:, 0, :], gu[:, 1, :])
                    prods.append(pr)
                for dt in range(Dt):
                    w2s = w2_p.tile([128, DFT * 128], BF16, tag="w2")
                    nc.sync.dma_start(
                        w2s.rearrange("p (j c) -> p j c", c=128),
                        w2[l, ch, dt].rearrange("j p c -> p j c"))
                    ps = ps_av.tile([128, S_], F32, tag="av")
                    for jj in range(DFT):
                        nc.tensor.matmul(ps[:],
                                         w2s[:, jj * 128:(jj + 1) * 128],
                                         prods[jj][:], start=(jj == 0),
                                         stop=(jj == DFT - 1))
                    nc.vector.tensor_add(x_sb[dt][:], x_sb[dt][:], ps[:])

        # ---------------- final rmsnorm ----------------
        rn = rmsnorm_bcast(x_sb)
        for dt in range(Dt):
            xn = tmp_p.tile([128, S_], F32, tag="xn")
            nc.vector.tensor_mul(xn[:], x_sb[dt][:], rn[:])
            nc.vector.tensor_scalar_mul(xn[:], xn[:], fw_sb[:, dt:dt + 1])
            nc.sync.dma_start(out[dt * 128:(dt + 1) * 128, :], xn[:])

    nc.compile()
    return nc


# ---------------------------------------------------------------------------
# host-side preparation
# ---------------------------------------------------------------------------

def _bf16(a):
    return np.ascontiguousarray(np.asarray(a, dtype=np.float32)).astype(
        ml_dtypes.bfloat16)


def _perm(HD_):
    """Head-dim pair permutation: slot 2i <- dim i, slot 2i+1 <- dim i+HD/2."""
    half = HD_ // 2
    p = np.empty(HD_, dtype=np.int64)
    p[0::2] = np.arange(half)
    p[1::2] = np.arange(half) + half
    return p


def _tok_idx(r, T_, S_):
    """Strided token ownership: core r of a group owns global q-tiles
    {r, r+4, r+8, r+12} (128 tokens each)."""
    NP = S_ // 128
    idx = np.empty(S_, dtype=np.int64)
    for p in range(NP):
        t = p * G + r
        idx[p * 128:(p + 1) * 128] = np.arange(t * 128, (t + 1) * 128)
    return idx


def prepare_in_maps(x0, cos, sin, wq, wk, wv_, wo_, anw, mnw, w1, w3, w2_,
                    fnw, T_, S_, D_, NH_, HD_, DFF_, L_, n_cores=8):
    """Build per-core input dicts. x0 is the already-expanded [B', T, D] fp32
    input (B' = n_cores // G batches)."""
    Dt = D_ // 128
    QT = (NH_ * HD_) // 128
    NP = S_ // 128
    CH = max(1, DFF_ // 1024)
    DFT = (DFF_ // CH) // 128
    perm = _perm(HD_)

    # fold norm weights into the consuming projections
    wq_e = anw[:, :, None] * wq      # [L, D, D]
    wk_e = anw[:, :, None] * wk
    wv_e = anw[:, :, None] * wv_
    w1_e = mnw[:, :, None] * w1      # [L, D, DFF]
    w3_e = mnw[:, :, None] * w3

    # permute q/k columns per head by `perm`
    def permute_cols(w):
        wh = w.reshape(L_, D_, NH_, HD_)
        return wh[:, :, :, perm].reshape(L_, D_, NH_ * HD_)

    wq_p = permute_cols(wq_e)
    wk_p = permute_cols(wk_e)

    # packed q|k lhsT tiles: [L, 2QT, Dt, 128, 128]
    wqk_pack = np.empty((L_, 2 * QT, Dt, 128, 128), dtype=np.float32)
    for j in range(QT):
        for kt in range(Dt):
            wqk_pack[:, j, kt] = wq_p[:, kt * 128:(kt + 1) * 128,
                                      j * 128:(j + 1) * 128]
            wqk_pack[:, QT + j, kt] = wk_p[:, kt * 128:(kt + 1) * 128,
                                           j * 128:(j + 1) * 128]
    # wo packed: [L, Dt, QT, 128, 128]; rows = o dims (head-major)
    wo_pack = np.empty((L_, Dt, QT, 128, 128), dtype=np.float32)
    for dt in range(Dt):
        for j in range(QT):
            wo_pack[:, dt, j] = wo_[:, j * 128:(j + 1) * 128,
                                    dt * 128:(dt + 1) * 128]
    # w13 packed: [L, CH, DFT, 2, Dt, 128, 128]
    csz = DFF_ // CH
    w13_pack = np.empty((L_, CH, DFT, 2, Dt, 128, 128), dtype=np.float32)
    for ch in range(CH):
        for df in range(DFT):
            c0 = ch * csz + df * 128
            for kt in range(Dt):
                w13_pack[:, ch, df, 0, kt] = w1_e[:, kt * 128:(kt + 1) * 128,
                                                  c0:c0 + 128]
                w13_pack[:, ch, df, 1, kt] = w3_e[:, kt * 128:(kt + 1) * 128,
                                                  c0:c0 + 128]
    # w2 packed: [L, CH, Dt, DFT, 128, 128]
    w2_pack = np.empty((L_, CH, Dt, DFT, 128, 128), dtype=np.float32)
    for ch in range(CH):
        for dt in range(Dt):
            for j in range(DFT):
                r0 = ch * csz + j * 128
                w2_pack[:, ch, dt, j] = w2_[:, r0:r0 + 128,
                                            dt * 128:(dt + 1) * 128]

    wqk_b = _bf16(wqk_pack)
    wv_b = _bf16(wv_e)
    wo_b = _bf16(wo_pack)
    w13_b = _bf16(w13_pack)
    w2_b = _bf16(w2_pack)
    fw_np = np.ascontiguousarray(
        np.asarray(fnw, np.float32).reshape(Dt, 128).T)  # [128, Dt]

    # rope tables, permuted + sign-baked, duplicated per head pair -> [128, T]
    cosPf = np.asarray(cos, np.float32)[:, perm].T        # [HD, T]
    sinf = np.asarray(sin, np.float32)[:, perm].T         # [HD, T]
    sign = np.where(np.arange(HD_) % 2 == 0, -1.0, 1.0)[:, None]
    sinPf = sinf * sign
    cosP2 = np.tile(cosPf, (2, 1))                        # [128, T]
    sinP2 = np.tile(sinPf, (2, 1))

    tril = np.tril(np.ones((128, 128), np.float32)).T  # mask[key, q] = key<=q

    in_maps = []
    for c in range(n_cores):
        b = c // G
        r = c % G
        tok = _tok_idx(r, T_, S_)
        xs = np.ascontiguousarray(x0[b, tok, :].T).astype(np.float32)
        # zone masks [128(key), p, i, k2, 128(q)]: slot 4p+2i+k2 vs q-tile
        # 4p+r: full below diagonal, tril on it, zero above.
        m = np.zeros((128, NP, 2, 2, 128), np.float32)
        for p in range(NP):
            for i in range(2):
                for k2 in range(2):
                    kidx = 2 * i + k2
                    if kidx < r:
                        m[:, p, i, k2, :] = 1.0
                    elif kidx == r:
                        m[:, p, i, k2, :] = tril
        in_maps.append({
            "x0T": xs,
            "wqk": wqk_b, "wv": wv_b, "wo": wo_b, "w13": w13_b, "w2": w2_b,
            "cosP": _bf16(cosP2[:, tok]),
            "sinP": _bf16(sinP2[:, tok]),
            "masks": m.astype(ml_dtypes.bfloat16),
            "fw": fw_np,
        })
    return in_maps


def expand_input(x_processed, boundaries, counts, x_residual):
    """Ragged chunk expansion: token t of batch b takes chunk
    #{boundaries[b] <= t}, plus residual."""
    xp = np.asarray(x_processed, np.float32)
    bd = np.asarray(boundaries)
    xr = np.asarray(x_residual, np.float32)
    Bn, Tn, Dn = xr.shape
    tt = np.arange(Tn)
    out = np.empty_like(xr)
    for b in range(Bn):
        idx = np.searchsorted(bd[b], tt, side="right")
        out[b] = xp[b, idx, :] + xr[b]
    return out


_NC_CACHE = {}


def _get_nc(key):
    if key not in _NC_CACHE:
        _NC_CACHE[key] = build_decoder(*key)
    return _NC_CACHE[key]


def kernel(x_processed, boundaries, counts, x_residual, cos, sin, seq_len,
           wq, wk, wv, wo, attn_norm_w, mlp_norm_w, w1, w3, w2, final_norm_w,
           _trace=False):
    S_ = T // G
    x0 = expand_input(x_processed, boundaries, counts, x_residual)
    in_maps = prepare_in_maps(
        x0, cos, sin,
        np.asarray(wq, np.float32), np.asarray(wk, np.float32),
        np.asarray(wv, np.float32), np.asarray(wo, np.float32),
        np.asarray(attn_norm_w, np.float32), np.asarray(mlp_norm_w, np.float32),
        np.asarray(w1, np.float32), np.asarray(w3, np.float32),
        np.asarray(w2, np.float32), np.asarray(final_norm_w, np.float32),
        T, S_, D, NH, HD, DFF, L, n_cores=8)
    nc = _get_nc((T, S_, D, NH, HD, DFF, L, 8))
    res = run_bass_kernel_spmd(nc, in_maps, list(range(8)), trace=_trace)
    outp = np.empty((B, T, D), dtype=np.float32)
    for c in range(8):
        b, r = c // G, c % G
        tok = _tok_idx(r, T, S_)
        outp[b, tok, :] = res.results[c]["out"].T
    if _trace:
        kernel.last_exec_time_ns = res.exec_time_ns
        kernel.last_results = res
    return outp
